# revision 16
# baseline (speedup 1.0000x reference)
"""Trainium2 Bass kernel for nn_AutoregressiveFeedback (B=256 data-parallel / 8 cores).

Pipeline: MHA self-attention -> 3-layer LSTM warmup scan -> autoregressive
2-cell LSTM decode -> scaled dot-product attention over predictions -> projection.

Per-core layout strategy (Bc = 32):
  * attention:  scores folded through G_h = (Wq_h Wk_h^T)/sqrt(KD) and the
    value/output projection through P_h = Wv_h Wo_h (host-side, weight-only).
    Scores are built transposed (S^T[k,q]) per head-PAIR (2 PSUM banks each,
    double-buffered so batch b+1's scores overlap batch b's exp); exp on
    ScalarE; A@V runs with a ones column appended so the softmax denominator
    lands in psum column 64.  The four normalized head pieces are summed on
    DVE (x is a true 64-dim signal) and one PE transpose per query tile
    produces xT for the LSTM.
  * LSTM: z stays in [batch, gates] orientation.  The three layers run as a
    wavefront (layer l at tick tau handles t = tau - l); each 64-unit gate
    chunk occupies one PE column-group (4 chunks x 32 batch rows = 128 psum
    partitions).  Gate columns are host-permuted to [i f o | g] per chunk and
    the g columns are pre-scaled by 2 so ONE sigmoid covers all 256 gate
    columns (tanh(z) = 2*sigmoid(2z) - 1, folded into the DVE ops via
    scalar_tensor_tensor); f*c runs on GpSimd.  Hidden state is transposed
    every tick by ONE DVE 32x32 block-transpose straight into SBUF; the
    resulting interleaved unit order is compensated by host-permuting the
    h-rows of every consuming weight matrix (HPERM).
    k-tile rounds are ordered so the latest-arriving hidden state feeds the
    final accumulation round.
  * decode: the linear feats() chain collapses to F' = Fw0 Fw1 Fw2, folded
    into cell-0's input weights (G = F' W0).  Cells 0/1 wavefront.  h1
    history is written straight into the pT archive that both the recurrence
    and the final attention read.
  * final attention: p p^T is symmetric so exp(scores) serves as its own
    transpose; the softmax denominator comes from activation accum_out.

All biases in this problem are zeros by construction (spec fill=zeros).
"""

import numpy as np
import ml_dtypes

import concourse.bass as bass
import concourse.bacc as bacc
import concourse.mybir as mybir
import concourse.tile as tile
from concourse.bass_utils import run_bass_kernel_spmd

BF = ml_dtypes.bfloat16
dt = mybir.dt
AF = mybir.ActivationFunctionType
ALU = mybir.AluOpType

B_FULL, FA, U, H, KD, NF = 256, 64, 256, 4, 64, 64
import os as _os
W_F32R = _os.environ.get("K_WF32R", "0") == "1"   # LSTM weights fp32r
H_F32 = _os.environ.get("K_HF32", "0") == "1"     # LSTM hidden state fp32
NCORES = 8
BC = B_FULL // NCORES  # 32

# unit order produced by the DVE 32x32 block-transpose of hS [128=4x32b, 64u]:
# k-tile j holds units {32j..32j+32} of each of the four 64-unit chunks
HPERM = np.concatenate([
    np.concatenate([np.arange(64 * c + 32 * j, 64 * c + 32 * j + 32)
                    for c in range(4)])
    for j in range(2)])


def _gate_perm(n_units, chunk):
    """Permute the 4*n_units gate columns so each `chunk`-unit block is
    laid out [i f o | g] (sigmoid prefix, tanh suffix)."""
    i0, f0, g0, o0 = 0, n_units, 2 * n_units, 3 * n_units
    cols = []
    for c in range(0, n_units, chunk):
        u = np.arange(c, c + chunk)
        cols.append(np.concatenate([g0 + u, i0 + u, f0 + u, o0 + u]))
    return np.concatenate(cols)


def _kt_split(w):
    """[K, N] -> [128, K//128, N] partition-major k-tiles."""
    K, N = w.shape
    assert K % 128 == 0
    return np.ascontiguousarray(w.reshape(K // 128, 128, N).transpose(1, 0, 2))


def build_host_tensors(inputs, T):
    f32 = np.float32
    g = lambda k: np.asarray(inputs[k], f32)
    Wq, Wk, Wv, Wo = g("Wq"), g("Wk"), g("Wv"), g("Wo")
    W0, U0, W1, U1, W2, U2 = g("W0"), g("U0"), g("W1"), g("U1"), g("W2"), g("U2")
    Fw0, Fw1, Fw2 = g("Fw0"), g("Fw1"), g("Fw2")
    pred_W = g("pred_W")
    x = g("inputs")
    ncores = x.shape[0] // BC

    gsb = np.zeros((128, 2, 64), f32)
    pcat = np.zeros((64, 256), f32)
    for h in range(H):
        Wq_h = Wq[:, h * KD:(h + 1) * KD]
        Wk_h = Wk[:, h * KD:(h + 1) * KD]
        Wv_h = Wv[:, h * KD:(h + 1) * KD]
        Wo_h = Wo[h * KD:(h + 1) * KD, :]
        G = (Wq_h @ Wk_h.T) / np.sqrt(KD)
        gsb[64 * (h % 2):64 * (h % 2) + 64, h // 2, :] = G
        pcat[:, h * 64:(h + 1) * 64] = Wv_h @ Wo_h
    pdup = np.concatenate([pcat, pcat], axis=0)

    permw = _gate_perm(U, 64)
    # g-gate columns pre-scaled by 2: tanh(z) = 2*sigmoid(2z) - 1
    gscale = np.tile(np.concatenate([np.full(64, 2.0, f32), np.ones(192, f32)]),
                     U // 64)

    def hp(m):
        return m[HPERM]

    def gw(m):
        return m[:, permw] * gscale[None, :]

    wmov = [
        _kt_split(gw(np.vstack([W0, np.zeros((64, 4 * U), f32), hp(U0)]))),
        _kt_split(gw(np.vstack([hp(W1), hp(U1)]))),
        _kt_split(gw(np.vstack([hp(W2), hp(U2)]))),
    ]
    Fp = Fw0 @ Fw1 @ Fw2
    wdec = [
        _kt_split(gw(np.vstack([hp(Fp @ W0), hp(U0)]))),
        _kt_split(gw(np.vstack([hp(W1), hp(U1)]))),
    ]
    WT = f32 if W_F32R else BF
    shared = {
        "wmov0x": np.ascontiguousarray(wmov[0][:, 0:1, :]).astype(BF),
        "gsb": gsb.astype(BF), "pdup": pdup.astype(BF),
        "wmov0": wmov[0].astype(WT), "wmov1": wmov[1].astype(WT),
        "wmov2": wmov[2].astype(WT),
        "wdec0": wdec[0].astype(WT), "wdec1": wdec[1].astype(WT),
        "predw": _kt_split(pred_W[HPERM]).astype(WT if H_F32 else BF),
        "eye": np.eye(128, dtype=f32).astype(BF),
        "eyef": np.eye(128, dtype=f32),
    }
    percore = []
    for c in range(ncores):
        xc = x[c * BC:(c + 1) * BC]
        inpT = np.ascontiguousarray(xc.transpose(2, 0, 1).reshape(FA, BC * T))
        percore.append({"inpT2": np.concatenate([inpT, inpT], 0).astype(BF)})
    return shared, percore


def build_program(T, S, attn_scale):
    BT = BC * T
    NT = BT // 128       # 128-row bt tiles
    KT = T // 128        # k tiles per sequence
    QT = T // 128
    WDT = dt.float32r if W_F32R else dt.bfloat16
    HDT = dt.float32 if H_F32 else dt.bfloat16
    PDT = dt.float32r if H_F32 else dt.bfloat16
    nc = bacc.Bacc("TRN2", target_bir_lowering=False, debug=False)

    d_inpT2 = nc.dram_tensor("inpT2", [128, BT], dt.bfloat16, kind="ExternalInput")
    d_gsb = nc.dram_tensor("gsb", [128, 2, 64], dt.bfloat16, kind="ExternalInput")
    d_pdup = nc.dram_tensor("pdup", [128, 256], dt.bfloat16, kind="ExternalInput")
    d_wmov = [nc.dram_tensor(f"wmov{l}", [128, 3 if l == 0 else 4, 1024], WDT,
                             kind="ExternalInput") for l in range(3)]
    d_wmov0x = nc.dram_tensor("wmov0x", [128, 1, 1024], dt.bfloat16,
                              kind="ExternalInput")
    d_wdec = [nc.dram_tensor(f"wdec{l}", [128, 4, 1024], WDT,
                             kind="ExternalInput") for l in range(2)]
    d_predw = nc.dram_tensor("predw", [128, 2, 64], PDT, kind="ExternalInput")
    d_eye = nc.dram_tensor("eye", [128, 128], dt.bfloat16, kind="ExternalInput")
    d_eyef = nc.dram_tensor("eyef", [128, 128], dt.float32, kind="ExternalInput")
    d_out = nc.dram_tensor("out", [BC, S, NF], dt.float32, kind="ExternalOutput")

    with tile.TileContext(nc) as tc:
        with tc.tile_pool(name="persist", bufs=1) as pp:
            eye_sb = pp.tile([128, 128], dt.bfloat16, tag="eye")
            nc.sync.dma_start(eye_sb[:], d_eye[:])
            eyeh_sb = eye_sb
            if H_F32:
                eyeh_sb = pp.tile([128, 128], dt.float32, tag="eyef")
                nc.sync.dma_start(eyeh_sb[:], d_eyef[:])
            predw_sb = pp.tile([128, 2, 64], PDT, tag="predw")
            nc.sync.dma_start(predw_sb[:], d_predw[:])
            xT4 = pp.tile([128, BT], dt.bfloat16, tag="xT4")
            nc.vector.memset(xT4[64:128, :], 0.0)
            pT = pp.tile([128, S, 2, 32], HDT, tag="pT")
            outf = pp.tile([S, BC * NF], dt.float32, tag="outf")

            # ================= attention =================
            with (
                tc.tile_pool(name="attn_sb", bufs=1) as asb,
                tc.tile_pool(name="attn_roll", bufs=3) as arl,
            ):
                inpT2 = asb.tile([128, BT], dt.bfloat16, tag="inpT2")
                nc.sync.dma_start(inpT2[:], d_inpT2[:])
                gsb = asb.tile([128, 2, 64], dt.bfloat16, tag="gsb")
                nc.sync.dma_start(gsb[:], d_gsb[:])
                pdup = asb.tile([128, 256], dt.bfloat16, tag="pdup")
                nc.sync.dma_start(pdup[:], d_pdup[:])
                w1T = [asb.tile([128, BT], dt.bfloat16, tag=f"w1T{i}", name=f"w1T{i}")
                       for i in range(2)]
                vE = asb.tile([128, NT, 4, 65], dt.bfloat16, tag="vE")
                nc.vector.memset(vE[:, :, :, 64], 1.0)

                # stage A: w1T_h = G_h^T @ inpT ; v'4 = inp @ [P_0..P_3]
                with tc.tile_pool(name="attn_psA", bufs=2, space="PSUM") as apsA:
                    for ntile in range(BT // 512):
                        cols = slice(ntile * 512, ntile * 512 + 512)
                        ps = [apsA.tile([128, 512], dt.float32, tag=f"w1ps{j}", name=f"w1ps{j}")
                              for j in range(2)]
                        for h in range(H):
                            r = 64 * (h % 2)
                            nc.tensor.matmul(
                                ps[h // 2][r:r + 64, :],
                                gsb[r:r + 64, h // 2, :],
                                inpT2[r:r + 64, cols],
                                skip_group_check=True)
                        for i in range(2):
                            if ntile % 2 == 0:
                                nc.vector.tensor_copy(w1T[i][:, cols], ps[i][:])
                            else:
                                nc.scalar.copy(w1T[i][:, cols], ps[i][:])
                    for nt2 in range(NT):
                        r = 64 * (nt2 % 2)
                        ps = apsA.tile([128, 256], dt.float32, tag="vps", bufs=4)
                        nc.tensor.matmul(
                            ps[:], inpT2[r:r + 64, nt2 * 128:nt2 * 128 + 128],
                            pdup[r:r + 64, :])
                        src = ps[:].rearrange("p (h d) -> p h d", h=4)
                        if nt2 % 2 == 0:
                            nc.vector.tensor_copy(vE[:, nt2, :, 0:64], src)
                        else:
                            nc.scalar.copy(vE[:, nt2, :, 0:64], src)

                # per-batch attention: head-pair halves pipeline through PSUM
                with (
                    tc.tile_pool(name="attn_psB", bufs=2, space="PSUM") as apsB,
                    tc.tile_pool(name="attn_psT", bufs=2, space="PSUM") as apsT,
                ):
                    for b in range(BC):
                        xs = arl.tile([128, QT, 64], dt.bfloat16, tag="xs")
                        for hh in range(2):
                            STh = apsB.tile([128, 2, KT, T], dt.float32, tag="STh")
                            for h2 in range(2):
                                r = 64 * h2
                                for kt in range(KT):
                                    nc.tensor.matmul(
                                        STh[:, h2, kt, :],
                                        inpT2[r:r + 64,
                                              b * T + kt * 128:b * T + kt * 128 + 128],
                                        w1T[hh][r:r + 64, b * T:b * T + T],
                                        skip_group_check=True)
                            expTh = arl.tile([128, 2, KT, T], dt.bfloat16, tag="expT")
                            nc.scalar.activation(expTh[:], STh[:], AF.Exp)
                            for qt in range(QT):
                                OP = apsB.tile([128, 2, 65], dt.float32, tag="OP")
                                with tc.tile_critical():
                                    i = 0
                                    for h2 in range(2):
                                        for kt in range(KT):
                                            nc.tensor.matmul(
                                                OP[:, h2, :],
                                                expTh[:, h2, kt, qt * 128:qt * 128 + 128],
                                                vE[:, b * KT + kt, 2 * hh + h2, :],
                                                start=(i == 0), stop=(i == 2 * KT - 1),
                                                skip_group_check=True)
                                            i += 1
                                rZ = arl.tile([128, 2], dt.float32, tag="rZ")
                                nc.vector.reciprocal(rZ[:], OP[:, :, 64])
                                xh = arl.tile([128, 2, 64], dt.bfloat16, tag="xh")
                                zb = bass.AP(rZ.tensor, rZ[:].offset,
                                             [rZ[:].ap[0], [1, 2], [0, 64]])
                                nc.vector.tensor_tensor(xh[:], OP[:, :, 0:64], zb,
                                                        ALU.mult)
                                if hh == 0:
                                    nc.vector.tensor_tensor(
                                        xs[:, qt, :], xh[:, 0, :], xh[:, 1, :], ALU.add)
                                else:
                                    nc.vector.tensor_tensor(
                                        xs[:, qt, :], xs[:, qt, :], xh[:, 0, :], ALU.add)
                                    nc.vector.tensor_tensor(
                                        xs[:, qt, :], xs[:, qt, :], xh[:, 1, :], ALU.add)
                        tp = apsT.tile([128, QT * 128], dt.bfloat16, tag="xTps")
                        for qt in range(QT):
                            nc.tensor.matmul(
                                tp[0:64, qt * 128:qt * 128 + 128],
                                xs[:, qt, :], eye_sb[:, 0:128],
                                is_transpose=True, skip_group_check=True)
                        nc.vector.tensor_copy(xT4[0:64, b * T:b * T + T], tp[0:64, :])

            # ================= LSTM phases =================
            with (
                tc.tile_pool(name="lstm_state", bufs=1) as lst,
                tc.tile_pool(name="lstm_roll", bufs=6) as lrl,
                tc.tile_pool(name="lstm_ps", bufs=2, space="PSUM") as lps,
            ):
                wmov_sb = []
                for l in range(3):
                    w = lst.tile([128, 3 if l == 0 else 4, 1024], WDT,
                                 tag=f"wmov{l}", name=f"wmov{l}")
                    nc.sync.dma_start(w[:], d_wmov[l][:])
                    wmov_sb.append(w)
                wdec_sb = []
                for l in range(2):
                    w = lst.tile([128, 4, 1024], WDT, tag=f"wdec{l}",
                                 name=f"wdec{l}")
                    nc.sync.dma_start(w[:], d_wdec[l][:])
                    wdec_sb.append(w)
                wmov0x_sb = lst.tile([128, 1, 1024], dt.bfloat16, tag="wmov0x")
                nc.sync.dma_start(wmov0x_sb[:], d_wmov0x[:])
                cS = lst.tile([128, 3, 64], dt.float32, tag="cS")
                hT = lst.tile([128, 6, 32], HDT, tag="hT")
                nc.vector.memset(cS[:], 0.0)
                nc.vector.memset(hT[:], 0.0)

                def cell_tick(Zp, Gs, T1, tcS, hS, fcS, slot, stats, rhss, hT_dst):
                    def cast_stat(ap):
                        if H_F32 and ap.dtype == dt.float32:
                            return ap.bitcast(dt.float32r)
                        return ap
                    nk = len(stats)
                    for c in range(4):
                        for kt in range(nk):
                            nc.tensor.matmul(
                                Zp[32 * c:32 * c + 32, slot, 0:256],
                                cast_stat(stats[kt]),
                                rhss[kt][:, 256 * c:256 * c + 256],
                                start=(kt == 0), stop=(kt == nk - 1),
                                tile_position=(0, 32 * c),
                                skip_group_check=True)
                    # gate cols are [g i | f o]; g cols hold 2*zg.
                    # split sigma so the (g,i) half lands first and T1 starts early
                    nc.scalar.activation(Gs[:, slot, 0:128], Zp[:, slot, 0:128],
                                         AF.Sigmoid)
                    nc.scalar.activation(Gs[:, slot, 128:256], Zp[:, slot, 128:256],
                                         AF.Sigmoid)
                    # T1 = (sigma(2zg) - 0.5) * i  ( = tanh(zg)*i/2 )
                    nc.vector.scalar_tensor_tensor(
                        T1[:, slot, :], Gs[:, slot, 0:64], 0.5,
                        Gs[:, slot, 64:128], ALU.subtract, ALU.mult)
                    nc.vector.tensor_tensor(fcS[:, slot, :], Gs[:, slot, 128:192],
                                            cS[:, slot, :], ALU.mult)
                    # c = f*c + 2*T1
                    nc.vector.scalar_tensor_tensor(
                        cS[:, slot, :], T1[:, slot, :], 2.0,
                        fcS[:, slot, :], ALU.mult, ALU.add)
                    nc.scalar.activation(tcS[:, slot, :], cS[:, slot, :], AF.Tanh)
                    nc.vector.tensor_tensor(hS[:, slot, :], Gs[:, slot, 192:256],
                                            tcS[:, slot, :], ALU.mult)
                    # 32x32 block transpose straight into SBUF (unit order HPERM)
                    nc.vector.transpose(hT_dst, hS[:, slot, :])

                # ---- warmup: 3-layer wavefront (emit L2, L1, L0 per tick) ----
                for tau in range(T + 2):
                    Zp = lps.tile([128, 3, 512], dt.float32, tag="Zp")
                    Gs = lrl.tile([128, 3, 256], dt.float32, tag="Gs")
                    T1 = lrl.tile([128, 3, 64], dt.float32, tag="T1w")
                    tcS = lrl.tile([128, 3, 64], dt.float32, tag="tcS")
                    hS = lrl.tile([128, 3, 64], HDT, tag="hS")
                    fcS = lrl.tile([128, 3, 64], dt.float32, tag="fcS")
                    for l in (2, 1, 0):
                        t = tau - l
                        if t < 0 or t >= T:
                            continue
                        wl = wmov_sb[l]
                        if l == 0:
                            stats = [xT4[:, t:BT:T], hT[:, 0, :], hT[:, 1, :]]
                            rhss = [wmov0x_sb[:, 0, :], wl[:, 1, :], wl[:, 2, :]]
                        elif l == 1:
                            # own h first (input h from cell0 lands latest)
                            stats = [hT[:, 2, :], hT[:, 3, :],
                                     hT[:, 0, :], hT[:, 1, :]]
                            rhss = [wl[:, 2, :], wl[:, 3, :],
                                    wl[:, 0, :], wl[:, 1, :]]
                        else:
                            stats = [hT[:, 4, :], hT[:, 5, :],
                                     hT[:, 2, :], hT[:, 3, :]]
                            rhss = [wl[:, 2, :], wl[:, 3, :],
                                    wl[:, 0, :], wl[:, 1, :]]
                        cell_tick(Zp, Gs, T1, tcS, hS, fcS, l, stats,
                                  rhss, hT[:, 2 * l:2 * l + 2, :])

                nc.vector.tensor_copy(pT[:, 0, :, :], hT[:, 4:6, :])

                # ---- decode: 2-cell wavefront (emit cell1, cell0 per tick) ----
                for tau in range(S):
                    Zp = lps.tile([128, 2, 512], dt.float32, tag="Zp")
                    Gs = lrl.tile([128, 2, 256], dt.float32, tag="Gsd")
                    T1 = lrl.tile([128, 2, 64], dt.float32, tag="T1d")
                    tcS = lrl.tile([128, 2, 64], dt.float32, tag="tcSd")
                    hS = lrl.tile([128, 2, 64], HDT, tag="hSd")
                    fcS = lrl.tile([128, 2, 64], dt.float32, tag="fcSd")
                    w1_ = tau           # cell1 computes step w1_
                    if 1 <= w1_ <= S - 1:
                        h1prev = ([hT[:, 2, :], hT[:, 3, :]] if w1_ == 1 else
                                  [pT[:, w1_ - 1, 0, :], pT[:, w1_ - 1, 1, :]])
                        # own h1 first, input h0 (lands latest) last
                        stats = h1prev + [hT[:, 0, :], hT[:, 1, :]]
                        rhss = [wdec_sb[1][:, k, :] for k in (2, 3, 0, 1)]
                        cell_tick(Zp, Gs, T1, tcS, hS, fcS, 1, stats,
                                  rhss, pT[:, w1_, :, :])
                    w0 = tau + 1        # cell0 computes step w0
                    if w0 <= S - 1:
                        # own h first; pT[w0-1] is written by cell1 THIS tick,
                        # so its two rounds must come last
                        stats = [hT[:, 0, :], hT[:, 1, :],
                                 pT[:, w0 - 1, 0, :], pT[:, w0 - 1, 1, :]]
                        rhss = [wdec_sb[0][:, k, :] for k in (2, 3, 0, 1)]
                        cell_tick(Zp, Gs, T1, tcS, hS, fcS, 0, stats,
                                  rhss, hT[:, 0:2, :])

            # ================= final attention over p =================
            with (
                tc.tile_pool(name="fin_roll", bufs=4) as frl,
                tc.tile_pool(name="fin_ps", bufs=2, space="PSUM") as fps,
            ):
                for b in range(BC):
                    ppps = fps.tile([S, 64], dt.float32, tag="ppps")
                    s2ps = fps.tile([S, S], dt.float32, tag="s2ps")
                    for kt in range(2):
                        pslice = pT[:, :, kt, b]   # [128, S] stride 64
                        if H_F32:
                            pslice = pslice.bitcast(dt.float32r)
                        nc.tensor.matmul(ppps[:], pslice, predw_sb[:, kt, :],
                                         start=(kt == 0), stop=(kt == 1))
                        nc.tensor.matmul(s2ps[:], pslice, pslice,
                                         start=(kt == 0), stop=(kt == 1))
                    expw = frl.tile([S, S], dt.bfloat16, tag="expw")
                    z2 = frl.tile([S, 1], dt.float32, tag="z2")
                    nc.scalar.activation(expw[:], s2ps[:], AF.Exp,
                                         scale=float(attn_scale),
                                         accum_out=z2[:])
                    ppsb = frl.tile([S, 64], dt.bfloat16, tag="ppsb")
                    nc.vector.tensor_copy(ppsb[:], ppps[:])
                    ops = fps.tile([S, 64], dt.float32, tag="ops")
                    nc.tensor.matmul(ops[:], expw[:], ppsb[:])
                    rz2 = frl.tile([S, 1], dt.float32, tag="rz2")
                    nc.vector.reciprocal(rz2[:], z2[:])
                    nc.vector.tensor_scalar(outf[:, b * NF:(b + 1) * NF], ops[:],
                                            rz2[:], None, ALU.mult)
                nc.sync.dma_start(
                    d_out[:].rearrange("b s f -> s b f"),
                    outf[:].rearrange("s (b f) -> s b f", b=BC))

    nc.compile()
    return nc


_cache = {}


def kernel(**inputs):
    x = np.asarray(inputs["inputs"])
    T = x.shape[1]
    S = 64
    attn_scale = float(np.asarray(inputs["attn_scale"]))
    ncores = x.shape[0] // BC

    shared, percore = build_host_tensors(inputs, T)
    key = (T, S, round(attn_scale, 9))
    if key not in _cache:
        _cache[key] = build_program(T, S, attn_scale)
    nc = _cache[key]

    in_maps = [dict(shared, **percore[c]) for c in range(ncores)]
    res = run_bass_kernel_spmd(nc, in_maps, list(range(ncores)))
    out = np.concatenate([res.results[c]["out"] for c in range(ncores)], axis=0)
    return np.ascontiguousarray(out.astype(np.float32))


# revision 24
# speedup vs baseline: 1.0616x; 1.0616x over previous
"""Trainium2 Bass kernel for nn_AutoregressiveFeedback (B=256 data-parallel / 8 cores).

Pipeline: MHA self-attention -> 3-layer LSTM warmup scan -> autoregressive
2-cell LSTM decode -> scaled dot-product attention over predictions -> projection.

Per-core layout strategy (Bc = 32):
  * attention:  scores folded through G_h = (Wq_h Wk_h^T)/sqrt(KD) and the
    value/output projection through P_h = Wv_h Wo_h (host-side, weight-only).
    Scores are built transposed (S^T[k,q]) per head-PAIR (2 PSUM banks each,
    double-buffered so batch b+1's scores overlap batch b's exp); exp on
    ScalarE; A@V runs with a ones column appended so the softmax denominator
    lands in psum column 64.  The four normalized head pieces are summed on
    DVE (x is a true 64-dim signal) and one PE transpose per query tile
    produces xT for the LSTM.
  * LSTM: z stays in [batch, gates] orientation.  The three layers run as a
    wavefront (layer l at tick tau handles t = tau - l); each 64-unit gate
    chunk occupies one PE column-group (4 chunks x 32 batch rows = 128 psum
    partitions).  Gate columns are host-permuted to [i f o | g] per chunk and
    the g columns are pre-scaled by 2 so ONE sigmoid covers all 256 gate
    columns (tanh(z) = 2*sigmoid(2z) - 1, folded into the DVE ops via
    scalar_tensor_tensor); f*c runs on GpSimd.  Hidden state is transposed
    every tick by ONE DVE 32x32 block-transpose straight into SBUF; the
    resulting interleaved unit order is compensated by host-permuting the
    h-rows of every consuming weight matrix (HPERM).
    k-tile rounds are ordered so the latest-arriving hidden state feeds the
    final accumulation round.
  * decode: the linear feats() chain collapses to F' = Fw0 Fw1 Fw2, folded
    into cell-0's input weights (G = F' W0).  Cells 0/1 wavefront.  h1
    history is written straight into the pT archive that both the recurrence
    and the final attention read.
  * final attention: p p^T is symmetric so exp(scores) serves as its own
    transpose; the softmax denominator comes from activation accum_out.

All biases in this problem are zeros by construction (spec fill=zeros).
"""

import numpy as np
import ml_dtypes

import concourse.bass as bass
import concourse.bacc as bacc
import concourse.mybir as mybir
import concourse.tile as tile
from concourse.bass_utils import run_bass_kernel_spmd

BF = ml_dtypes.bfloat16
dt = mybir.dt
AF = mybir.ActivationFunctionType
ALU = mybir.AluOpType

B_FULL, FA, U, H, KD, NF = 256, 64, 256, 4, 64, 64
import os as _os
W_F32R = _os.environ.get("K_WF32R", "0") == "1"   # LSTM weights fp32r
H_F32 = _os.environ.get("K_HF32", "0") == "1"     # LSTM hidden state fp32
NCORES = 8
BC = B_FULL // NCORES  # 32

# unit order produced by the DVE 32x32 block-transpose of hS [128=4x32b, 64u]:
# k-tile j holds units {32j..32j+32} of each of the four 64-unit chunks
HPERM = np.concatenate([
    np.concatenate([np.arange(64 * c + 32 * j, 64 * c + 32 * j + 32)
                    for c in range(4)])
    for j in range(2)])


def _gate_perm(n_units, chunk):
    """Permute the 4*n_units gate columns so each `chunk`-unit block is
    laid out [i f o | g] (sigmoid prefix, tanh suffix)."""
    i0, f0, g0, o0 = 0, n_units, 2 * n_units, 3 * n_units
    cols = []
    for c in range(0, n_units, chunk):
        u = np.arange(c, c + chunk)
        cols.append(np.concatenate([i0 + u, f0 + u, o0 + u, g0 + u]))
    return np.concatenate(cols)


def _kt_split(w):
    """[K, N] -> [128, K//128, N] partition-major k-tiles."""
    K, N = w.shape
    assert K % 128 == 0
    return np.ascontiguousarray(w.reshape(K // 128, 128, N).transpose(1, 0, 2))


def build_host_tensors(inputs, T):
    f32 = np.float32
    g = lambda k: np.asarray(inputs[k], f32)
    Wq, Wk, Wv, Wo = g("Wq"), g("Wk"), g("Wv"), g("Wo")
    W0, U0, W1, U1, W2, U2 = g("W0"), g("U0"), g("W1"), g("U1"), g("W2"), g("U2")
    Fw0, Fw1, Fw2 = g("Fw0"), g("Fw1"), g("Fw2")
    pred_W = g("pred_W")
    x = g("inputs")
    ncores = x.shape[0] // BC

    gsb = np.zeros((128, 2, 64), f32)
    pcat = np.zeros((64, 256), f32)
    for h in range(H):
        Wq_h = Wq[:, h * KD:(h + 1) * KD]
        Wk_h = Wk[:, h * KD:(h + 1) * KD]
        Wv_h = Wv[:, h * KD:(h + 1) * KD]
        Wo_h = Wo[h * KD:(h + 1) * KD, :]
        G = (Wq_h @ Wk_h.T) / np.sqrt(KD)
        gsb[64 * (h % 2):64 * (h % 2) + 64, h // 2, :] = G
        pcat[:, h * 64:(h + 1) * 64] = Wv_h @ Wo_h
    pdup = np.concatenate([pcat, pcat], axis=0)

    permw = _gate_perm(U, 64)
    # g-gate columns pre-scaled by 2: tanh(z) = 2*sigmoid(2z) - 1
    gscale = np.tile(np.concatenate([np.ones(192, f32), np.full(64, 2.0, f32)]),
                     U // 64)

    def hp(m):
        return m[HPERM]

    def gw(m):
        return m[:, permw] * gscale[None, :]

    wmov = [
        _kt_split(gw(np.vstack([W0, np.zeros((64, 4 * U), f32), hp(U0)]))),
        _kt_split(gw(np.vstack([hp(W1), hp(U1)]))),
        _kt_split(gw(np.vstack([hp(W2), hp(U2)]))),
    ]
    Fp = Fw0 @ Fw1 @ Fw2
    wdec = [
        _kt_split(gw(np.vstack([hp(Fp @ W0), hp(U0)]))),
        _kt_split(gw(np.vstack([hp(W1), hp(U1)]))),
    ]
    WT = f32 if W_F32R else BF
    shared = {
        "wmov0x": np.ascontiguousarray(wmov[0][:, 0:1, :]).astype(BF),
        "gsb": gsb.astype(BF), "pdup": pdup.astype(BF),
        "wmov0": wmov[0].astype(WT), "wmov1": wmov[1].astype(WT),
        "wmov2": wmov[2].astype(WT),
        "wdec0": wdec[0].astype(WT), "wdec1": wdec[1].astype(WT),
        "predw": _kt_split(pred_W[HPERM]).astype(WT if H_F32 else BF),
        "eye": np.eye(128, dtype=f32).astype(BF),
        "eyef": np.eye(128, dtype=f32),
    }
    percore = []
    for c in range(ncores):
        xc = x[c * BC:(c + 1) * BC]
        inpT = np.ascontiguousarray(xc.transpose(2, 0, 1).reshape(FA, BC * T))
        percore.append({"inpT2": np.concatenate([inpT, inpT], 0).astype(BF)})
    return shared, percore


def build_program(T, S, attn_scale):
    BT = BC * T
    NT = BT // 128       # 128-row bt tiles
    KT = T // 128        # k tiles per sequence
    QT = T // 128
    WDT = dt.float32r if W_F32R else dt.bfloat16
    HDT = dt.float32 if H_F32 else dt.bfloat16
    PDT = dt.float32r if H_F32 else dt.bfloat16
    nc = bacc.Bacc("TRN2", target_bir_lowering=False, debug=False)

    d_inpT2 = nc.dram_tensor("inpT2", [128, BT], dt.bfloat16, kind="ExternalInput")
    d_gsb = nc.dram_tensor("gsb", [128, 2, 64], dt.bfloat16, kind="ExternalInput")
    d_pdup = nc.dram_tensor("pdup", [128, 256], dt.bfloat16, kind="ExternalInput")
    d_wmov = [nc.dram_tensor(f"wmov{l}", [128, 3 if l == 0 else 4, 1024], WDT,
                             kind="ExternalInput") for l in range(3)]
    d_wmov0x = nc.dram_tensor("wmov0x", [128, 1, 1024], dt.bfloat16,
                              kind="ExternalInput")
    d_wdec = [nc.dram_tensor(f"wdec{l}", [128, 4, 1024], WDT,
                             kind="ExternalInput") for l in range(2)]
    d_predw = nc.dram_tensor("predw", [128, 2, 64], PDT, kind="ExternalInput")
    d_eye = nc.dram_tensor("eye", [128, 128], dt.bfloat16, kind="ExternalInput")
    d_eyef = nc.dram_tensor("eyef", [128, 128], dt.float32, kind="ExternalInput")
    d_out = nc.dram_tensor("out", [BC, S, NF], dt.float32, kind="ExternalOutput")

    with tile.TileContext(nc) as tc:
        with tc.tile_pool(name="persist", bufs=1) as pp:
            eye_sb = pp.tile([128, 128], dt.bfloat16, tag="eye")
            nc.sync.dma_start(eye_sb[:], d_eye[:])
            eyeh_sb = eye_sb
            if H_F32:
                eyeh_sb = pp.tile([128, 128], dt.float32, tag="eyef")
                nc.sync.dma_start(eyeh_sb[:], d_eyef[:])
            predw_sb = pp.tile([128, 2, 64], PDT, tag="predw")
            nc.sync.dma_start(predw_sb[:], d_predw[:])
            xT4 = pp.tile([128, BT], dt.bfloat16, tag="xT4")
            nc.vector.memset(xT4[64:128, :], 0.0)
            pT = pp.tile([128, S, 2, 32], HDT, tag="pT")
            outf = pp.tile([S, BC * NF], dt.float32, tag="outf")

            # ================= attention =================
            with (
                tc.tile_pool(name="attn_sb", bufs=1) as asb,
                tc.tile_pool(name="attn_roll", bufs=3) as arl,
            ):
                inpT2 = asb.tile([128, BT], dt.bfloat16, tag="inpT2")
                nc.sync.dma_start(inpT2[:], d_inpT2[:])
                gsb = asb.tile([128, 2, 64], dt.bfloat16, tag="gsb")
                nc.sync.dma_start(gsb[:], d_gsb[:])
                pdup = asb.tile([128, 256], dt.bfloat16, tag="pdup")
                nc.sync.dma_start(pdup[:], d_pdup[:])
                w1T = [asb.tile([128, BT], dt.bfloat16, tag=f"w1T{i}", name=f"w1T{i}")
                       for i in range(2)]
                vE = asb.tile([128, NT, 4, 65], dt.bfloat16, tag="vE")
                nc.vector.memset(vE[:, :, :, 64], 1.0)

                # stage A: w1T_h = G_h^T @ inpT ; v'4 = inp @ [P_0..P_3]
                with tc.tile_pool(name="attn_psA", bufs=2, space="PSUM") as apsA:
                    for ntile in range(BT // 512):
                        cols = slice(ntile * 512, ntile * 512 + 512)
                        ps = [apsA.tile([128, 512], dt.float32, tag=f"w1ps{j}", name=f"w1ps{j}")
                              for j in range(2)]
                        for h in range(H):
                            r = 64 * (h % 2)
                            nc.tensor.matmul(
                                ps[h // 2][r:r + 64, :],
                                gsb[r:r + 64, h // 2, :],
                                inpT2[r:r + 64, cols],
                                skip_group_check=True)
                        for i in range(2):
                            if ntile % 2 == 0:
                                nc.vector.tensor_copy(w1T[i][:, cols], ps[i][:])
                            else:
                                nc.scalar.copy(w1T[i][:, cols], ps[i][:])
                    for nt2 in range(NT):
                        r = 64 * (nt2 % 2)
                        ps = apsA.tile([128, 256], dt.float32, tag="vps", bufs=4)
                        nc.tensor.matmul(
                            ps[:], inpT2[r:r + 64, nt2 * 128:nt2 * 128 + 128],
                            pdup[r:r + 64, :])
                        src = ps[:].rearrange("p (h d) -> p h d", h=4)
                        if nt2 % 2 == 0:
                            nc.vector.tensor_copy(vE[:, nt2, :, 0:64], src)
                        else:
                            nc.scalar.copy(vE[:, nt2, :, 0:64], src)

                # per-batch attention: head-pair halves pipeline through PSUM
                with (
                    tc.tile_pool(name="attn_psB", bufs=2, space="PSUM") as apsB,
                    tc.tile_pool(name="attn_psT", bufs=2, space="PSUM") as apsT,
                ):
                    for b in range(BC):
                        xs = arl.tile([128, QT, 64], dt.bfloat16, tag="xs")
                        for hh in range(2):
                            STh = apsB.tile([128, 2, KT, T], dt.float32, tag="STh")
                            for h2 in range(2):
                                r = 64 * h2
                                for kt in range(KT):
                                    nc.tensor.matmul(
                                        STh[:, h2, kt, :],
                                        inpT2[r:r + 64,
                                              b * T + kt * 128:b * T + kt * 128 + 128],
                                        w1T[hh][r:r + 64, b * T:b * T + T],
                                        skip_group_check=True)
                            expTh = arl.tile([128, 2, KT, T], dt.bfloat16, tag="expT")
                            nc.scalar.activation(expTh[:], STh[:], AF.Exp)
                            for qt in range(QT):
                                OP = apsB.tile([128, 2, 65], dt.float32, tag="OP")
                                with tc.tile_critical():
                                    i = 0
                                    for h2 in range(2):
                                        for kt in range(KT):
                                            nc.tensor.matmul(
                                                OP[:, h2, :],
                                                expTh[:, h2, kt, qt * 128:qt * 128 + 128],
                                                vE[:, b * KT + kt, 2 * hh + h2, :],
                                                start=(i == 0), stop=(i == 2 * KT - 1),
                                                skip_group_check=True)
                                            i += 1
                                rZ = arl.tile([128, 2], dt.float32, tag="rZ")
                                nc.vector.reciprocal(rZ[:], OP[:, :, 64])
                                xh = arl.tile([128, 2, 64], dt.bfloat16, tag="xh")
                                zb = bass.AP(rZ.tensor, rZ[:].offset,
                                             [rZ[:].ap[0], [1, 2], [0, 64]])
                                nc.vector.tensor_tensor(xh[:], OP[:, :, 0:64], zb,
                                                        ALU.mult)
                                if hh == 0:
                                    nc.vector.tensor_tensor(
                                        xs[:, qt, :], xh[:, 0, :], xh[:, 1, :], ALU.add)
                                else:
                                    nc.vector.tensor_tensor(
                                        xs[:, qt, :], xs[:, qt, :], xh[:, 0, :], ALU.add)
                                    nc.vector.tensor_tensor(
                                        xs[:, qt, :], xs[:, qt, :], xh[:, 1, :], ALU.add)
                        tp = apsT.tile([128, QT * 128], dt.bfloat16, tag="xTps")
                        for qt in range(QT):
                            nc.tensor.matmul(
                                tp[0:64, qt * 128:qt * 128 + 128],
                                xs[:, qt, :], eye_sb[:, 0:128],
                                is_transpose=True, skip_group_check=True)
                        nc.vector.tensor_copy(xT4[0:64, b * T:b * T + T], tp[0:64, :])

            # ================= LSTM phases =================
            with (
                tc.tile_pool(name="lstm_state", bufs=1) as lst,
                tc.tile_pool(name="lstm_roll", bufs=6) as lrl,
                tc.tile_pool(name="lstm_ps", bufs=2, space="PSUM") as lps,
            ):
                wmov_sb = []
                for l in range(3):
                    w = lst.tile([128, 3 if l == 0 else 4, 1024], WDT,
                                 tag=f"wmov{l}", name=f"wmov{l}")
                    nc.sync.dma_start(w[:], d_wmov[l][:])
                    wmov_sb.append(w)
                wdec_sb = []
                for l in range(2):
                    w = lst.tile([128, 4, 1024], WDT, tag=f"wdec{l}",
                                 name=f"wdec{l}")
                    nc.sync.dma_start(w[:], d_wdec[l][:])
                    wdec_sb.append(w)
                wmov0x_sb = lst.tile([128, 1, 1024], dt.bfloat16, tag="wmov0x")
                nc.sync.dma_start(wmov0x_sb[:], d_wmov0x[:])
                cS = lst.tile([128, 3, 64], dt.float32, tag="cS")
                hT = lst.tile([128, 6, 32], HDT, tag="hT")
                nc.vector.memset(cS[:], 0.0)
                nc.vector.memset(hT[:], 0.0)

                def cell_tick(Zp, Gs, T1, tcS, hS, fcS, slot, stats, rhss, hT_dst):
                    def cast_stat(ap):
                        if H_F32 and ap.dtype == dt.float32:
                            return ap.bitcast(dt.float32r)
                        return ap
                    nk = len(stats)
                    for c in range(4):
                        for kt in range(nk):
                            nc.tensor.matmul(
                                Zp[32 * c:32 * c + 32, slot, 0:256],
                                cast_stat(stats[kt]),
                                rhss[kt][:, 256 * c:256 * c + 256],
                                start=(kt == 0), stop=(kt == nk - 1),
                                tile_position=(0, 32 * c),
                                skip_group_check=True)
                    # one sigmoid over all 256 gate cols; g cols hold sigma(2*zg)
                    nc.scalar.activation(Gs[:, slot, :], Zp[:, slot, 0:256],
                                         AF.Sigmoid)
                    # T1 = (sigma(2zg) - 0.5) * i  ( = tanh(zg)*i/2 )
                    nc.vector.scalar_tensor_tensor(
                        T1[:, slot, :], Gs[:, slot, 192:256], 0.5,
                        Gs[:, slot, 0:64], ALU.subtract, ALU.mult)
                    nc.vector.tensor_tensor(fcS[:, slot, :], Gs[:, slot, 64:128],
                                            cS[:, slot, :], ALU.mult)
                    # c = f*c + 2*T1
                    nc.vector.scalar_tensor_tensor(
                        cS[:, slot, :], T1[:, slot, :], 2.0,
                        fcS[:, slot, :], ALU.mult, ALU.add)
                    nc.scalar.activation(tcS[:, slot, :], cS[:, slot, :], AF.Tanh)
                    nc.vector.tensor_tensor(hS[:, slot, :], Gs[:, slot, 128:192],
                                            tcS[:, slot, :], ALU.mult)
                    # 32x32 block transpose straight into SBUF (unit order HPERM)
                    nc.vector.transpose(hT_dst, hS[:, slot, :])

                # ---- warmup: 3-layer wavefront (emit L2, L1, L0 per tick) ----
                for tau in range(T + 2):
                    Zp = lps.tile([128, 3, 512], dt.float32, tag="Zp")
                    # HAM-warmth filler: keep the PE array streaming through
                    # chain stalls (output is never read; scratch bank)
                    dz = lps.tile([128, 512], dt.float32, tag="dz")
                    for dk in range(2):
                        nc.tensor.matmul(dz[:], wmov0x_sb[:, 0, 0:128],
                                         wmov_sb[1][:, dk, 0:512],
                                         start=(dk == 0), stop=(dk == 1),
                                         skip_group_check=True)
                    Gs = lrl.tile([128, 3, 256], dt.float32, tag="Gs")
                    T1 = lrl.tile([128, 3, 64], dt.float32, tag="T1w")
                    tcS = lrl.tile([128, 3, 64], dt.float32, tag="tcS")
                    hS = lrl.tile([128, 3, 64], HDT, tag="hS")
                    fcS = lrl.tile([128, 3, 64], dt.float32, tag="fcS")
                    for l in (2, 1, 0):
                        t = tau - l
                        if t < 0 or t >= T:
                            continue
                        wl = wmov_sb[l]
                        if l == 0:
                            stats = [xT4[:, t:BT:T], hT[:, 0, :], hT[:, 1, :]]
                            rhss = [wmov0x_sb[:, 0, :], wl[:, 1, :], wl[:, 2, :]]
                        elif l == 1:
                            # own h first (input h from cell0 lands latest)
                            stats = [hT[:, 2, :], hT[:, 3, :],
                                     hT[:, 0, :], hT[:, 1, :]]
                            rhss = [wl[:, 2, :], wl[:, 3, :],
                                    wl[:, 0, :], wl[:, 1, :]]
                        else:
                            stats = [hT[:, 4, :], hT[:, 5, :],
                                     hT[:, 2, :], hT[:, 3, :]]
                            rhss = [wl[:, 2, :], wl[:, 3, :],
                                    wl[:, 0, :], wl[:, 1, :]]
                        cell_tick(Zp, Gs, T1, tcS, hS, fcS, l, stats,
                                  rhss, hT[:, 2 * l:2 * l + 2, :])

                nc.vector.tensor_copy(pT[:, 0, :, :], hT[:, 4:6, :])

                # ---- decode: 2-cell wavefront (emit cell1, cell0 per tick) ----
                for tau in range(S):
                    Zp = lps.tile([128, 2, 512], dt.float32, tag="Zp")
                    dz = lps.tile([128, 512], dt.float32, tag="dz")
                    for dk in range(2):
                        nc.tensor.matmul(dz[:], wmov0x_sb[:, 0, 0:128],
                                         wdec_sb[1][:, dk, 0:512],
                                         start=(dk == 0), stop=(dk == 1),
                                         skip_group_check=True)
                    Gs = lrl.tile([128, 2, 256], dt.float32, tag="Gsd")
                    T1 = lrl.tile([128, 2, 64], dt.float32, tag="T1d")
                    tcS = lrl.tile([128, 2, 64], dt.float32, tag="tcSd")
                    hS = lrl.tile([128, 2, 64], HDT, tag="hSd")
                    fcS = lrl.tile([128, 2, 64], dt.float32, tag="fcSd")
                    w1_ = tau           # cell1 computes step w1_
                    if 1 <= w1_ <= S - 1:
                        h1prev = ([hT[:, 2, :], hT[:, 3, :]] if w1_ == 1 else
                                  [pT[:, w1_ - 1, 0, :], pT[:, w1_ - 1, 1, :]])
                        # own h1 first, input h0 (lands latest) last
                        stats = h1prev + [hT[:, 0, :], hT[:, 1, :]]
                        rhss = [wdec_sb[1][:, k, :] for k in (2, 3, 0, 1)]
                        cell_tick(Zp, Gs, T1, tcS, hS, fcS, 1, stats,
                                  rhss, pT[:, w1_, :, :])
                    w0 = tau + 1        # cell0 computes step w0
                    if w0 <= S - 1:
                        # own h first; pT[w0-1] is written by cell1 THIS tick,
                        # so its two rounds must come last
                        stats = [hT[:, 0, :], hT[:, 1, :],
                                 pT[:, w0 - 1, 0, :], pT[:, w0 - 1, 1, :]]
                        rhss = [wdec_sb[0][:, k, :] for k in (2, 3, 0, 1)]
                        cell_tick(Zp, Gs, T1, tcS, hS, fcS, 0, stats,
                                  rhss, hT[:, 0:2, :])

            # ================= final attention over p =================
            with (
                tc.tile_pool(name="fin_roll", bufs=4) as frl,
                tc.tile_pool(name="fin_ps", bufs=2, space="PSUM") as fps,
            ):
                for b in range(BC):
                    ppps = fps.tile([S, 64], dt.float32, tag="ppps")
                    s2ps = fps.tile([S, S], dt.float32, tag="s2ps")
                    for kt in range(2):
                        pslice = pT[:, :, kt, b]   # [128, S] stride 64
                        if H_F32:
                            pslice = pslice.bitcast(dt.float32r)
                        nc.tensor.matmul(ppps[:], pslice, predw_sb[:, kt, :],
                                         start=(kt == 0), stop=(kt == 1))
                        nc.tensor.matmul(s2ps[:], pslice, pslice,
                                         start=(kt == 0), stop=(kt == 1))
                    expw = frl.tile([S, S], dt.bfloat16, tag="expw")
                    z2 = frl.tile([S, 1], dt.float32, tag="z2")
                    nc.scalar.activation(expw[:], s2ps[:], AF.Exp,
                                         scale=float(attn_scale),
                                         accum_out=z2[:])
                    ppsb = frl.tile([S, 64], dt.bfloat16, tag="ppsb")
                    nc.vector.tensor_copy(ppsb[:], ppps[:])
                    ops = fps.tile([S, 64], dt.float32, tag="ops")
                    nc.tensor.matmul(ops[:], expw[:], ppsb[:])
                    rz2 = frl.tile([S, 1], dt.float32, tag="rz2")
                    nc.vector.reciprocal(rz2[:], z2[:])
                    nc.vector.tensor_scalar(outf[:, b * NF:(b + 1) * NF], ops[:],
                                            rz2[:], None, ALU.mult)
                nc.sync.dma_start(
                    d_out[:].rearrange("b s f -> s b f"),
                    outf[:].rearrange("s (b f) -> s b f", b=BC))

    nc.compile()
    return nc


_cache = {}


def kernel(**inputs):
    x = np.asarray(inputs["inputs"])
    T = x.shape[1]
    S = 64
    attn_scale = float(np.asarray(inputs["attn_scale"]))
    ncores = x.shape[0] // BC

    shared, percore = build_host_tensors(inputs, T)
    key = (T, S, round(attn_scale, 9))
    if key not in _cache:
        _cache[key] = build_program(T, S, attn_scale)
    nc = _cache[key]

    in_maps = [dict(shared, **percore[c]) for c in range(ncores)]
    res = run_bass_kernel_spmd(nc, in_maps, list(range(ncores)))
    out = np.concatenate([res.results[c]["out"] for c in range(ncores)], axis=0)
    return np.ascontiguousarray(out.astype(np.float32))
